# revision 9
# baseline (speedup 1.0000x reference)
"""Trainium2 Bass kernel for nn_MultiHeadAttention_88192858456426.

Reference computation (per batch b, C=512 channels, N=2048 tokens):
    qp = wq @ q + bq          # [C, N]
    kp = wk @ k + bk          # [C, N]
    vp = wv @ v + bv          # [C, N]
    S[m, n]  = sum_c kp[c, m] * qp[c, n]        # QK^T (transposed view)
    out[c,n] = sum_m vp[c, m] * S[m, n] + q[c, n]

Algebraic restructure (removes the whole qp projection from the device):
    S = q^T (Wq^T Wk) k  +  alpha[n]  +  beta[m]  +  gamma
with G = Wq^T Wk precomputed on the host (it is input-data independent),
alpha = q^T (Wq^T bk), beta = k^T (Wk^T bq), gamma = bq.bk.  On device:
    Gk[X,m] = sum_j G[X,j] k[j,m] + u[X]      (u = Wq^T bk; the +u row
              makes the S contraction emit alpha[n] for free)
    beta'[m] = sum_j k[j,m] w[j] + gamma      (w = Wk^T bq; 64 thin
              matmuls, folded into the S PSUM->SBUF copy as a bias)
    S~[m,n] = sum_X Gk[X,m] q[X,n] + beta'[m]   (q used RAW - no qp!)
    out[c,n] = sum_m vp[c,m] S~[m,n] + q[c,n]

Sharding: data-parallel over batch B=8 across the 8 NeuronCores (one batch
per core, no collectives).

All matmuls are fp16 operands (1 col/cycle, weight loads hidden), fp32
PSUM accumulation. Streaming-cycle budget per core @2.4GHz:
    Gk 32768 + beta 8192 + vpt 32768 + S 131072 + out 131072 = 335.9Kc
    = 139.9us (vs 150.2us for the baseline with an explicit qp phase).
The out accumulation is 2-pass (c01 while S streams, c23 after) on every
block, which frees 2 PSUM banks vs 4 live accumulators and overlaps half
the residual+store tail with matmuls.
"""

import numpy as np
from contextlib import ExitStack

import concourse.bass as bass
import concourse.mybir as mybir
import concourse.tile as tile
from concourse import bacc
from concourse.bass_utils import run_bass_kernel_spmd

P = 128            # partitions
C = 512            # channels
N = 2048           # tokens
NB = 512           # n-block width (one PSUM bank of fp32)
CK = C // P        # 4 channel chunks
MCH = N // P       # 16 token chunks
NBK = N // NB      # 4 n-blocks

F32 = mybir.dt.float32
FP16 = mybir.dt.float16
ACT_IDENT = mybir.ActivationFunctionType.Identity

N_CORES = 8
MODE = "fp16"      # matmul operand dtype (kept for test.py compat)
HOST_BETA = True  # True: beta'[m] comes in as a host-computed input


def build_nc(reps=1, mode="fp16"):
    MDT = FP16
    nc = bacc.Bacc("TRN2", target_bir_lowering=False, debug=False,
                   num_devices=N_CORES)

    q_d = nc.dram_tensor("q", [C, N], MDT, kind="ExternalInput").ap()
    k_d = nc.dram_tensor("k", [C, N], MDT, kind="ExternalInput").ap()
    v_d = nc.dram_tensor("v", [C, N], MDT, kind="ExternalInput").ap()
    gT_d = nc.dram_tensor("gT", [C, C], MDT, kind="ExternalInput").ap()
    wvT_d = nc.dram_tensor("wvT", [C, C], MDT, kind="ExternalInput").ap()
    ugt_d = nc.dram_tensor("ugt", [P, CK], F32, kind="ExternalInput").ap()
    if HOST_BETA:
        bsb_d = nc.dram_tensor("bsb_in", [P, MCH], F32,
                               kind="ExternalInput").ap()
    else:
        wcol_d = nc.dram_tensor("wcol", [P, CK], MDT,
                                kind="ExternalInput").ap()
        gcol_d = nc.dram_tensor("gcol", [P, 1], F32,
                                kind="ExternalInput").ap()
    bvb_d = nc.dram_tensor("bvb", [P, C], F32, kind="ExternalInput").ap()
    o_d = nc.dram_tensor("o", [C, N], F32, kind="ExternalOutput").ap()

    with ExitStack() as ctx:
        tc = ctx.enter_context(tile.TileContext(nc))
        consts = ctx.enter_context(tc.tile_pool(name="consts", bufs=1))
        wpool = ctx.enter_context(tc.tile_pool(name="wpool", bufs=1))
        kvraw = ctx.enter_context(tc.tile_pool(name="kvraw", bufs=3))
        persist = ctx.enter_context(tc.tile_pool(name="persist", bufs=1))
        qpool = ctx.enter_context(tc.tile_pool(name="qpool", bufs=2))
        s16 = ctx.enter_context(tc.tile_pool(name="s16", bufs=MCH))
        opool = ctx.enter_context(tc.tile_pool(name="opool", bufs=4))
        ps_a = ctx.enter_context(tc.tile_pool(name="ps_a", bufs=2, space="PSUM"))
        ps_s = ctx.enter_context(tc.tile_pool(name="ps_s", bufs=3, space="PSUM"))
        ps_r = ctx.enter_context(tc.tile_pool(name="ps_r", bufs=2, space="PSUM"))

        for rep in range(reps):
            # ---- phase A: Gk[X, m] = G k + u, kept in SBUF; beta' row ----
            # interleave gT chunk i with the first k quarter so the first
            # accumulation group's operands arrive in issue order
            gT_sb, kq0 = [], []
            for i in range(CK):
                t = wpool.tile([P, C], MDT, tag=f"gt{i}", name=f"gt{i}")
                nc.sync.dma_start(t[:], gT_d[i * P:(i + 1) * P, :])
                gT_sb.append(t)
                t = kvraw.tile([P, NB], MDT, tag=f"kv{i}", name=f"kv{i}")
                nc.scalar.dma_start(t[:], k_d[i * P:(i + 1) * P, 0:NB])
                kq0.append(t)
            ugt = consts.tile([P, CK], F32, tag="ugt", name="ugt")
            nc.sync.dma_start(ugt[:], ugt_d[:])
            if not HOST_BETA:
                wcol = consts.tile([P, CK], MDT, tag="wcol", name="wcol")
                nc.sync.dma_start(wcol[:], wcol_d[:])
                gcol = consts.tile([P, 1], F32, tag="gcol", name="gcol")
                nc.sync.dma_start(gcol[:], gcol_d[:])

            gk_sb = [persist.tile([P, N], MDT, tag=f"gk{c}", name=f"gk{c}")
                     for c in range(CK)]
            vpt_sb = [persist.tile([P, C], MDT, tag=f"vpt{m}", name=f"vpt{m}")
                      for m in range(MCH)]
            bsb = consts.tile([P, MCH], F32, tag="bsb", name="bsb")
            if HOST_BETA:
                nc.sync.dma_start(bsb[:], bsb_d[:])

            def emit_vpt_quarter(hq, vq):
                for ml in range(NB // P):
                    m = hq * (NB // P) + ml
                    ps = ps_s.tile([P, C], F32, tag="ps_s", name="ps_s")
                    for i in range(CK):
                        nc.tensor.matmul(
                            ps[:],
                            vq[i][:, ml * P:(ml + 1) * P],
                            wv_sb[i][:],
                            start=(i == 0), stop=(i == CK - 1))
                    nc.vector.tensor_add(vpt_sb[m][:], ps[:], bvb[:])
            for hq in range(NBK):
                if hq == 0:
                    kq = kq0
                else:
                    kq = []
                    for i in range(CK):
                        t = kvraw.tile([P, NB], MDT, tag=f"kv{i}",
                                       name=f"kv{i}")
                        nc.sync.dma_start(
                            t[:], k_d[i * P:(i + 1) * P,
                                      hq * NB:(hq + 1) * NB])
                        kq.append(t)
                if hq == 2:
                    # phase-B criticals queue behind the third k quarter,
                    # matching the order the PE consumes them
                    wv_sb, vq0 = [], []
                    for i in range(CK):
                        t = wpool.tile([P, C], MDT, tag=f"wv{i}", name=f"wv{i}")
                        nc.sync.dma_start(t[:], wvT_d[i * P:(i + 1) * P, :])
                        wv_sb.append(t)
                    bvb = consts.tile([P, C], F32, tag="bvb", name="bvb")
                    nc.sync.dma_start(bvb[:], bvb_d[:])
                    for i in range(CK):
                        t = kvraw.tile([P, NB], MDT, tag=f"kv{i}",
                                       name=f"kv{i}")
                        nc.sync.dma_start(t[:], v_d[i * P:(i + 1) * P, 0:NB])
                        vq0.append(t)
                for c in range(CK):
                    ps = ps_a.tile([P, NB], F32, tag="ps_a", name="ps_a")
                    for i in range(CK):
                        nc.tensor.matmul(
                            ps[:],
                            gT_sb[i][:, c * P:(c + 1) * P],
                            kq[i][:],
                            start=(i == 0), stop=(i == CK - 1))
                    nc.scalar.activation(
                        gk_sb[c][:, hq * NB:(hq + 1) * NB],
                        ps[:], ACT_IDENT, bias=ugt[:, c:c + 1])
                if not HOST_BETA:
                    # beta'[m] for this quarter's 4 m-chunks: thin matmuls
                    # out[m-part, 1] = sum_i k[i, m] w[i]
                    bps = ps_a.tile([P, NB // P], F32, tag="bps", name="bps",
                                    bufs=1)
                    for ml in range(NB // P):
                        for i in range(CK):
                            nc.tensor.matmul(
                                bps[:, ml:ml + 1],
                                kq[i][:, ml * P:(ml + 1) * P],
                                wcol[:, i:i + 1],
                                start=(i == 0), stop=(i == CK - 1))
                    nc.scalar.activation(
                        bsb[:, hq * (NB // P):(hq + 1) * (NB // P)],
                        bps[:], ACT_IDENT, bias=gcol[:])
                # interleave the first two vpt quarters into phase A so the
                # PE alternates k- and v-dependent work while DMA catches up
                if hq == 2:
                    emit_vpt_quarter(0, vq0)
                if hq == 3:
                    vq1 = []
                    for i in range(CK):
                        t = kvraw.tile([P, NB], MDT, tag=f"kv{i}",
                                       name=f"kv{i}")
                        nc.sync.dma_start(t[:], v_d[i * P:(i + 1) * P,
                                                    NB:2 * NB])
                        vq1.append(t)
                    emit_vpt_quarter(1, vq1)

            # ---- phase B remainder: vpt quarters 2 and 3 ----
            for hq in (2, 3):
                vq = []
                for i in range(CK):
                    t = kvraw.tile([P, NB], MDT, tag=f"kv{i}", name=f"kv{i}")
                    nc.sync.dma_start(t[:], v_d[i * P:(i + 1) * P,
                                                hq * NB:(hq + 1) * NB])
                    vq.append(t)
                if hq == 3:
                    # prefetch q block 0 so phase C starts without a DMA wait
                    qt_cur = []
                    for i in range(CK):
                        t = qpool.tile([P, NB], MDT, tag=f"qt{i}",
                                       name=f"qt{i}")
                        nc.sync.dma_start(
                            t[:], q_d[i * P:(i + 1) * P, 0:NB])
                        qt_cur.append(t)
                emit_vpt_quarter(hq, vq)

            # ---- phase C: per n-block: S (+beta bias), out 2-pass ----
            for bi in range(NBK):
                b0, w = bi * NB, NB
                qt = qt_cur
                if bi + 1 < NBK:
                    n0 = (bi + 1) * NB
                    qt_cur = []
                    for i in range(CK):
                        t = qpool.tile([P, NB], MDT, tag=f"qt{i}",
                                       name=f"qt{i}")
                        nc.sync.dma_start(
                            t[:], q_d[i * P:(i + 1) * P, n0:n0 + NB])
                        qt_cur.append(t)

                def emit_s(m):
                    ps = ps_s.tile([P, w], F32, tag="ps_s", name="ps_s")
                    for c in range(CK):
                        nc.tensor.matmul(
                            ps[:],
                            gk_sb[c][:, m * P:(m + 1) * P],
                            qt[c][:],
                            start=(c == 0), stop=(c == CK - 1))
                    return ps

                def emit_out(c, r_ps_c):
                    o_sb = opool.tile([P, w], F32, tag="o", name="o")
                    nc.vector.tensor_add(o_sb[:], r_ps_c[:], qt[c][:])
                    eng = nc.sync if c % 2 == 0 else nc.scalar
                    eng.dma_start(o_d[c * P:(c + 1) * P, b0:b0 + w],
                                  o_sb[:])

                # pass 1: c-chunks 0/1 accumulate as S tiles are produced;
                # pass 2: c-chunks 2/3 re-stream the kept S tiles, so half
                # the residual+store tail overlaps pass-2 matmuls
                r_ps = {cc: ps_r.tile([P, w], F32, tag="ps_r", name="ps_r")
                        for cc in (0, 1)}
                s_keep = []
                s_ps_prev = emit_s(0)
                for m in range(MCH):
                    s_ps_next = emit_s(m + 1) if m + 1 < MCH else None
                    s_sb = s16.tile([P, w], MDT, tag="sl", name="sl")
                    # PSUM->SBUF copy folds in the beta'[m] bias; alternate
                    # ACT and DVE to balance engine load
                    if m % 2 == 0:
                        nc.scalar.activation(s_sb[:], s_ps_prev[:],
                                             ACT_IDENT,
                                             bias=bsb[:, m:m + 1])
                    else:
                        nc.vector.tensor_scalar_add(s_sb[:], s_ps_prev[:],
                                                    bsb[:, m:m + 1])
                    s_keep.append(s_sb)
                    for c in (0, 1):
                        nc.tensor.matmul(
                            r_ps[c][:],
                            vpt_sb[m][:, c * P:(c + 1) * P],
                            s_sb[:],
                            start=(m == 0), stop=(m == MCH - 1))
                    s_ps_prev = s_ps_next
                emit_out(0, r_ps[0])
                emit_out(1, r_ps[1])
                for c in (2, 3):
                    r = ps_r.tile([P, w], F32, tag="ps_r", name="ps_r")
                    for m in range(MCH):
                        nc.tensor.matmul(
                            r[:],
                            vpt_sb[m][:, c * P:(c + 1) * P],
                            s_keep[m][:],
                            start=(m == 0), stop=(m == MCH - 1))
                    emit_out(c, r)

    nc.finalize()
    return nc


_CACHE = {}


def _get_nc():
    if "nc" not in _CACHE:
        _CACHE["nc"] = build_nc(mode=MODE)
    return _CACHE["nc"]


def _in_maps(q, k, v, wq, bq, wk, bk, wv, bv, mode=None):
    f32 = lambda x: np.ascontiguousarray(np.asarray(x), dtype=np.float32)
    h16 = lambda x: np.ascontiguousarray(np.asarray(x), dtype=np.float16)
    wq, bq = np.asarray(wq, np.float32), np.asarray(bq, np.float32)
    wk, bk = np.asarray(wk, np.float32), np.asarray(bk, np.float32)
    q, k, v = h16(q), h16(k), h16(v)
    gT = h16(wk.T @ wq)            # lhsT[j, X] = G[X, j], G = wq.T @ wk
    wvT = h16(np.asarray(wv).T)
    u = wq.T @ bk                  # folds alpha[n] into Gk
    w_ = wk.T @ bq                 # beta'[m] = k^T w_ + gamma
    gam = float(bq @ bk)
    ugt = f32(u.reshape(CK, P).T)
    bvb = f32(np.tile(np.asarray(bv, np.float32)[None, :], (P, 1)))
    if HOST_BETA:
        kf = np.asarray(k, np.float32)
        return [
            {"q": q[i], "k": k[i], "v": v[i],
             "gT": gT, "wvT": wvT, "ugt": ugt, "bvb": bvb,
             "bsb_in": f32((kf[i].T @ w_ + gam).reshape(MCH, P).T)}
            for i in range(N_CORES)
        ]
    wcol = h16(w_.reshape(CK, P).T)
    gcol = f32(np.full((P, 1), gam))
    return [
        {"q": q[i], "k": k[i], "v": v[i],
         "gT": gT, "wvT": wvT,
         "ugt": ugt, "wcol": wcol, "gcol": gcol, "bvb": bvb}
        for i in range(N_CORES)
    ]


def run(inputs, **spmd_kwargs):
    """Run on hardware; returns (output [B,C,N], BassKernelResults)."""
    nc = _get_nc()
    maps = _in_maps(**inputs)
    res = run_bass_kernel_spmd(nc, maps, list(range(N_CORES)), **spmd_kwargs)
    out = np.stack([res.results[i]["o"] for i in range(N_CORES)], axis=0)
    return out, res


def kernel(q, k, v, wq, bq, wk, bk, wv, bv):
    out, _ = run(dict(q=q, k=k, v=v, wq=wq, bq=bq, wk=wk, bk=bk,
                      wv=wv, bv=bv))
    return out


# revision 37
# speedup vs baseline: 1.3521x; 1.3521x over previous
"""Trainium2 Bass kernel for nn_MultiHeadAttention_88192858456426.

Reference computation (per batch b, C=512 channels, N=2048 tokens):
    qp = wq @ q + bq          # [C, N]
    kp = wk @ k + bk          # [C, N]
    vp = wv @ v + bv          # [C, N]
    S[m, n]  = sum_c kp[c, m] * qp[c, n]        # QK^T (transposed view)
    out[c,n] = sum_m vp[c, m] * S[m, n] + q[c, n]

Algebraic restructure (removes the whole qp projection from the device):
    S = q^T (Wq^T Wk) k  +  alpha[n]  +  beta[m]  +  gamma
with G = Wq^T Wk precomputed on the host (it is input-data independent),
alpha = q^T (Wq^T bk), beta = k^T (Wk^T bq), gamma = bq.bk.  On device:
    Gk[X,m] = sum_j G[X,j] k[j,m] + u[X]      (u = Wq^T bk; the +u row
              makes the S contraction emit alpha[n] for free)
    beta'[m] = sum_j k[j,m] w[j] + gamma      (w = Wk^T bq, computed on
              device - see BETA_IMPL - and folded into the S PSUM->SBUF
              copy as a per-partition ACT/DVE bias)
    S~[m,n] = sum_X Gk[X,m] q[X,n] + beta'[m]   (q used RAW - no qp!)
    out[c,n] = sum_m vp[c,m] S~[m,n] + q[c,n]

Only weight-derived quantities (G, u, w, gamma, transposes/bias tilings)
are precomputed on the host; all per-token compute stays on device.

Sharding: data-parallel over batch B=8 across the 8 NeuronCores (one batch
per core, no collectives).

All matmuls are fp16 operands (1 col/cycle, weight loads hidden), fp32
PSUM accumulation. Streaming-cycle budget per core @2.4GHz:
    Gk 32768 + beta 8192 + vpt 32768 + S 131072 + out 131072 = 335.9Kc
    = 139.9us (vs 150.2us for the baseline with an explicit qp phase).
The out accumulation is 2-pass (c01 while S streams, c23 after) on every
block, which frees 2 PSUM banks vs 4 live accumulators and overlaps half
the residual+store tail with matmuls.
"""

import numpy as np
from contextlib import ExitStack

import concourse.bass as bass
import concourse.mybir as mybir
import concourse.tile as tile
from concourse import bacc
from concourse.bass_utils import run_bass_kernel_spmd

P = 128            # partitions
C = 512            # channels
N = 2048           # tokens
NB = 512           # n-block width (one PSUM bank of fp32)
CK = C // P        # 4 channel chunks
MCH = N // P       # 16 token chunks
NBK = N // NB      # 4 n-blocks

F32 = mybir.dt.float32
FP16 = mybir.dt.float16
ACT_IDENT = mybir.ActivationFunctionType.Identity

N_CORES = 8
MODE = "fp16"      # matmul operand dtype (kept for test.py compat)
HOST_BETA = False  # True: beta'[m] comes in as a host-computed input
# beta' device impl: "thin" = 64 [128x128]-lhsT 1-col matmuls (loads exposed),
# "row" = 16 1-col-lhsT matmuls into a [1,N] psum row (loads ~free) + 4 PE
# transposes into the [P, MCH] bias layout (fewer PE instruction slots)
BETA_IMPL = "row"


def build_nc(reps=1, mode="fp16"):
    MDT = FP16
    nc = bacc.Bacc("TRN2", target_bir_lowering=False, debug=False,
                   num_devices=N_CORES)

    q_d = nc.dram_tensor("q", [C, N], MDT, kind="ExternalInput").ap()
    k_d = nc.dram_tensor("k", [C, N], MDT, kind="ExternalInput").ap()
    v_d = nc.dram_tensor("v", [C, N], MDT, kind="ExternalInput").ap()
    gT_d = nc.dram_tensor("gT", [C, C], MDT, kind="ExternalInput").ap()
    wvT_d = nc.dram_tensor("wvT", [C, C], MDT, kind="ExternalInput").ap()
    ugt_d = nc.dram_tensor("ugt", [P, CK], F32, kind="ExternalInput").ap()
    if HOST_BETA:
        bsb_d = nc.dram_tensor("bsb_in", [P, MCH], F32,
                               kind="ExternalInput").ap()
    else:
        wcol_d = nc.dram_tensor("wcol", [P, CK], MDT,
                                kind="ExternalInput").ap()
        gcol_d = nc.dram_tensor("gcol", [P, 1], F32,
                                kind="ExternalInput").ap()
        if BETA_IMPL == "row":
            id4_d = nc.dram_tensor("id4", [NBK, NBK], F32,
                                   kind="ExternalInput").ap()
    bvb_d = nc.dram_tensor("bvb", [P, C], F32, kind="ExternalInput").ap()
    o_d = nc.dram_tensor("o", [C, N], F32, kind="ExternalOutput").ap()

    with ExitStack() as ctx:
        tc = ctx.enter_context(tile.TileContext(nc))
        consts = ctx.enter_context(tc.tile_pool(name="consts", bufs=1))
        wpool = ctx.enter_context(tc.tile_pool(name="wpool", bufs=1))
        kvraw = ctx.enter_context(tc.tile_pool(name="kvraw", bufs=4))
        persist = ctx.enter_context(tc.tile_pool(name="persist", bufs=1))
        qpool = ctx.enter_context(tc.tile_pool(name="qpool", bufs=3))
        s16 = ctx.enter_context(tc.tile_pool(name="s16", bufs=MCH))
        opool = ctx.enter_context(tc.tile_pool(name="opool", bufs=4))
        ps_a = ctx.enter_context(tc.tile_pool(name="ps_a", bufs=2, space="PSUM"))
        ps_s = ctx.enter_context(tc.tile_pool(name="ps_s", bufs=3, space="PSUM"))
        ps_r = ctx.enter_context(tc.tile_pool(name="ps_r", bufs=2, space="PSUM"))

        for rep in range(reps):
            # ---- phase A: Gk[X, m] = G k + u, kept in SBUF; beta' row ----
            # interleave gT chunk i with the first k quarter so the first
            # accumulation group's operands arrive in issue order
            gT_sb, kq0 = [], []
            for i in range(CK):
                t = wpool.tile([P, C], MDT, tag=f"gt{i}", name=f"gt{i}")
                nc.sync.dma_start(t[:], gT_d[i * P:(i + 1) * P, :])
                gT_sb.append(t)
                t = kvraw.tile([P, NB], MDT, tag=f"kv{i}", name=f"kv{i}")
                # all input loads ride the SP DMA queue; stores ride ACT, so cross-rep
                # input prefetch never queues behind the previous rep's output writes
                nc.sync.dma_start(t[:], k_d[i * P:(i + 1) * P, 0:NB])
                kq0.append(t)
            ugt = consts.tile([P, CK], F32, tag="ugt", name="ugt")
            nc.sync.dma_start(ugt[:], ugt_d[:])
            if not HOST_BETA:
                wcol = consts.tile([P, CK], MDT, tag="wcol", name="wcol")
                nc.sync.dma_start(wcol[:], wcol_d[:])
                gcol = consts.tile([P, 1], F32, tag="gcol", name="gcol")
                nc.sync.dma_start(gcol[:], gcol_d[:])
                if BETA_IMPL == "row":
                    id4 = consts.tile([NBK, NBK], F32, tag="id4", name="id4")
                    nc.sync.dma_start(id4[:], id4_d[:])
                    brow = consts.tile([NBK, NB], F32, tag="brow",
                                       name="brow")

            gk_sb = [persist.tile([P, N], MDT, tag=f"gk{c}", name=f"gk{c}")
                     for c in range(CK)]
            vpt_sb = [persist.tile([P, C], MDT, tag=f"vpt{m}", name=f"vpt{m}")
                      for m in range(MCH)]
            bsb = consts.tile([P, MCH], F32, tag="bsb", name="bsb")
            if HOST_BETA:
                nc.sync.dma_start(bsb[:], bsb_d[:])

            def emit_vpt_quarter(hq, vq):
                for ml in range(NB // P):
                    m = hq * (NB // P) + ml
                    ps = ps_s.tile([P, C], F32, tag="ps_s", name="ps_s")
                    for i in range(CK):
                        nc.tensor.matmul(
                            ps[:],
                            vq[i][:, ml * P:(ml + 1) * P],
                            wv_sb[i][:],
                            start=(i == 0), stop=(i == CK - 1))
                    nc.vector.tensor_add(vpt_sb[m][:], ps[:], bvb[:])
            for hq in range(NBK):
                if hq == 0:
                    kq = kq0
                else:
                    kq = []
                    for i in range(CK):
                        t = kvraw.tile([P, NB], MDT, tag=f"kv{i}",
                                       name=f"kv{i}")
                        nc.sync.dma_start(
                            t[:], k_d[i * P:(i + 1) * P,
                                      hq * NB:(hq + 1) * NB])
                        kq.append(t)
                if hq == 2:
                    # phase-B criticals queue behind the third k quarter,
                    # matching the order the PE consumes them
                    wv_sb, vq0 = [], []
                    for i in range(CK):
                        t = wpool.tile([P, C], MDT, tag=f"wv{i}", name=f"wv{i}")
                        nc.sync.dma_start(t[:], wvT_d[i * P:(i + 1) * P, :])
                        wv_sb.append(t)
                    bvb = consts.tile([P, C], F32, tag="bvb", name="bvb")
                    nc.sync.dma_start(bvb[:], bvb_d[:])
                    for i in range(CK):
                        t = kvraw.tile([P, NB], MDT, tag=f"kv{i}",
                                       name=f"kv{i}")
                        nc.sync.dma_start(t[:], v_d[i * P:(i + 1) * P,
                                                      0:NB])
                        vq0.append(t)
                for c in range(CK):
                    ps = ps_a.tile([P, NB], F32, tag="ps_a", name="ps_a")
                    for i in range(CK):
                        nc.tensor.matmul(
                            ps[:],
                            gT_sb[i][:, c * P:(c + 1) * P],
                            kq[i][:],
                            start=(i == 0), stop=(i == CK - 1))
                    nc.scalar.activation(
                        gk_sb[c][:, hq * NB:(hq + 1) * NB],
                        ps[:], ACT_IDENT, bias=ugt[:, c:c + 1])
                if not HOST_BETA and BETA_IMPL == "thin":
                    # beta'[m] for this quarter's 4 m-chunks: thin matmuls
                    # out[m-part, 1] = sum_i k[i, m] w[i]
                    bps = ps_a.tile([P, NB // P], F32, tag="bps", name="bps",
                                    bufs=1)
                    for ml in range(NB // P):
                        for i in range(CK):
                            nc.tensor.matmul(
                                bps[:, ml:ml + 1],
                                kq[i][:, ml * P:(ml + 1) * P],
                                wcol[:, i:i + 1],
                                start=(i == 0), stop=(i == CK - 1))
                    nc.scalar.activation(
                        bsb[:, hq * (NB // P):(hq + 1) * (NB // P)],
                        bps[:], ACT_IDENT, bias=gcol[:])
                if not HOST_BETA and BETA_IMPL == "row":
                    # beta' row for this quarter: 1-col lhsT (w chunk), k as
                    # the moving operand -> [1, NB] psum row, +gamma on copy
                    bps = ps_a.tile([1, NB], F32, tag="bps", name="bps",
                                    bufs=1)
                    for i in range(CK):
                        nc.tensor.matmul(
                            bps[:], wcol[:, i:i + 1], kq[i][:],
                            start=(i == 0), stop=(i == CK - 1))
                    # engines can't write at a partition offset; bounce the
                    # psum row through SBUF at partition 0, then DMA it to
                    # partition hq of brow
                    btmp = consts.tile([1, NB], F32, tag="btmp", name="btmp",
                                       bufs=2)
                    nc.scalar.copy(btmp[:], bps[:])
                    nc.sync.dma_start(brow[hq:hq + 1, :], btmp[:])
                # interleave the first two vpt quarters into phase A so the
                # PE alternates k- and v-dependent work while DMA catches up
                if hq == 2:
                    emit_vpt_quarter(0, vq0)
                if hq == 3:
                    vq1 = []
                    for i in range(CK):
                        t = kvraw.tile([P, NB], MDT, tag=f"kv{i}",
                                       name=f"kv{i}")
                        nc.sync.dma_start(t[:], v_d[i * P:(i + 1) * P,
                                                      NB:2 * NB])
                        vq1.append(t)
                    emit_vpt_quarter(1, vq1)

            # ---- phase B remainder: vpt quarters 2 and 3 ----
            for hq in (2, 3):
                vq = []
                for i in range(CK):
                    t = kvraw.tile([P, NB], MDT, tag=f"kv{i}", name=f"kv{i}")
                    nc.sync.dma_start(t[:], v_d[i * P:(i + 1) * P,
                                                  hq * NB:(hq + 1) * NB])
                    vq.append(t)
                if hq == 3:
                    # prefetch q block 0 so phase C starts without a DMA wait
                    qt_cur = []
                    for i in range(CK):
                        t = qpool.tile([P, NB], MDT, tag=f"qt{i}",
                                       name=f"qt{i}")
                        nc.sync.dma_start(
                            t[:], q_d[i * P:(i + 1) * P, 0:NB])
                        qt_cur.append(t)
                emit_vpt_quarter(hq, vq)

            if not HOST_BETA and BETA_IMPL == "row":
                # transpose [NBK, 128] slices of brow into [128, NBK] psum
                # tiles (bsb column order becomes ml*NBK + hq).  Emitted
                # after the last vpt quarter so the brow DMA chain finishes
                # in the shadow of phase-B matmuls instead of stalling the
                # PE at the phase-A tail.  The tps tiles share the "bps"
                # psum slot, whose row accumulations are all drained here.
                for ml in range(NB // P):
                    tps = ps_a.tile([P, NBK], F32, tag="bps", name="bts",
                                    bufs=1)
                    nc.tensor.transpose(
                        tps[:], brow[:, ml * P:(ml + 1) * P], id4[:])
                    nc.scalar.activation(
                        bsb[:, ml * NBK:(ml + 1) * NBK],
                        tps[:], ACT_IDENT, bias=gcol[:])

            # ---- phase C: per n-block: S (+beta bias), out 2-pass ----
            for bi in range(NBK):
                b0, w = bi * NB, NB
                qt = qt_cur
                if bi + 1 < NBK:
                    n0 = (bi + 1) * NB
                    qt_cur = []
                    for i in range(CK):
                        t = qpool.tile([P, NB], MDT, tag=f"qt{i}",
                                       name=f"qt{i}")
                        nc.sync.dma_start(
                            t[:], q_d[i * P:(i + 1) * P, n0:n0 + NB])
                        qt_cur.append(t)

                def emit_s(m):
                    ps = ps_s.tile([P, w], F32, tag="ps_s", name="ps_s")
                    for c in range(CK):
                        nc.tensor.matmul(
                            ps[:],
                            gk_sb[c][:, m * P:(m + 1) * P],
                            qt[c][:],
                            start=(c == 0), stop=(c == CK - 1))
                    return ps

                def emit_out(c, r_ps_c):
                    o_sb = opool.tile([P, w], F32, tag="o", name="o")
                    nc.vector.tensor_add(o_sb[:], r_ps_c[:], qt[c][:])
                    nc.scalar.dma_start(o_d[c * P:(c + 1) * P, b0:b0 + w],
                                        o_sb[:])

                # pass 1: c-chunks 0/1 accumulate as S tiles are produced;
                # pass 2: c-chunks 2/3 re-stream the kept S tiles, so half
                # the residual+store tail overlaps pass-2 matmuls
                r_ps = {cc: ps_r.tile([P, w], F32, tag="ps_r", name="ps_r")
                        for cc in (0, 1)}
                s_keep = []
                s_ps_prev = emit_s(0)
                for m in range(MCH):
                    s_ps_next = emit_s(m + 1) if m + 1 < MCH else None
                    s_sb = s16.tile([P, w], MDT, tag="sl", name="sl")
                    # PSUM->SBUF copy folds in the beta'[m] bias; alternate
                    # ACT and DVE to balance engine load
                    # device-row beta lands in bsb with permuted columns;
                    # host-provided beta is in natural m order
                    bm = ((m % 4) * NBK + m // 4) \
                        if (BETA_IMPL == "row" and not HOST_BETA) else m
                    if m % 2 == 0:
                        nc.scalar.activation(s_sb[:], s_ps_prev[:],
                                             ACT_IDENT,
                                             bias=bsb[:, bm:bm + 1])
                    else:
                        nc.vector.tensor_scalar_add(s_sb[:], s_ps_prev[:],
                                                    bsb[:, bm:bm + 1])
                    s_keep.append(s_sb)
                    for c in (0, 1):
                        nc.tensor.matmul(
                            r_ps[c][:],
                            vpt_sb[m][:, c * P:(c + 1) * P],
                            s_sb[:],
                            start=(m == 0), stop=(m == MCH - 1))
                    s_ps_prev = s_ps_next
                emit_out(0, r_ps[0])
                emit_out(1, r_ps[1])
                for c in (2, 3):
                    r = ps_r.tile([P, w], F32, tag="ps_r", name="ps_r")
                    for m in range(MCH):
                        nc.tensor.matmul(
                            r[:],
                            vpt_sb[m][:, c * P:(c + 1) * P],
                            s_keep[m][:],
                            start=(m == 0), stop=(m == MCH - 1))
                    emit_out(c, r)

    nc.finalize()
    return nc


_CACHE = {}


def _get_nc():
    if "nc" not in _CACHE:
        _CACHE["nc"] = build_nc(mode=MODE)
    return _CACHE["nc"]


def _in_maps(q, k, v, wq, bq, wk, bk, wv, bv, mode=None):
    f32 = lambda x: np.ascontiguousarray(np.asarray(x), dtype=np.float32)
    h16 = lambda x: np.ascontiguousarray(np.asarray(x), dtype=np.float16)
    wq, bq = np.asarray(wq, np.float32), np.asarray(bq, np.float32)
    wk, bk = np.asarray(wk, np.float32), np.asarray(bk, np.float32)
    q, k, v = h16(q), h16(k), h16(v)
    gT = h16(wk.T @ wq)            # lhsT[j, X] = G[X, j], G = wq.T @ wk
    wvT = h16(np.asarray(wv).T)
    u = wq.T @ bk                  # folds alpha[n] into Gk
    w_ = wk.T @ bq                 # beta'[m] = k^T w_ + gamma
    gam = float(bq @ bk)
    ugt = f32(u.reshape(CK, P).T)
    bvb = f32(np.tile(np.asarray(bv, np.float32)[None, :], (P, 1)))
    if HOST_BETA:
        kf = np.asarray(k, np.float32)
        return [
            {"q": q[i], "k": k[i], "v": v[i],
             "gT": gT, "wvT": wvT, "ugt": ugt, "bvb": bvb,
             "bsb_in": f32((kf[i].T @ w_ + gam).reshape(MCH, P).T)}
            for i in range(N_CORES)
        ]
    wcol = h16(w_.reshape(CK, P).T)
    gcol = f32(np.full((P, 1), gam))
    extra = {}
    if BETA_IMPL == "row":
        extra["id4"] = f32(np.eye(NBK))
    return [
        {"q": q[i], "k": k[i], "v": v[i],
         "gT": gT, "wvT": wvT,
         "ugt": ugt, "wcol": wcol, "gcol": gcol, "bvb": bvb, **extra}
        for i in range(N_CORES)
    ]


def run(inputs, **spmd_kwargs):
    """Run on hardware; returns (output [B,C,N], BassKernelResults)."""
    nc = _get_nc()
    maps = _in_maps(**inputs)
    res = run_bass_kernel_spmd(nc, maps, list(range(N_CORES)), **spmd_kwargs)
    out = np.stack([res.results[i]["o"] for i in range(N_CORES)], axis=0)
    return out, res


def kernel(q, k, v, wq, bq, wk, bk, wv, bv):
    out, _ = run(dict(q=q, k=k, v=v, wq=wq, bq=bq, wk=wk, bk=bk,
                      wv=wv, bv=bv))
    return out

